# revision 8
# baseline (speedup 1.0000x reference)
"""NetVLAD Trainium2 Bass kernel, SPMD over 8 NeuronCores.

Contract: kernel(x, Wc, C) takes the FULL inputs
  x  [16, 56, 56, 512] f32, Wc [512, 32] f32, C [512, 32] f32
and returns the FULL output [16, 16384] f32 (matches reference()).

Sharding: data-parallel over batch — 2 samples per core; Wc/C replicated.

v3 design:
  - x is uploaded TWICE, both in bf16: pixel-major xb [6272, 512] (moving
    operand of mm2) and host-pre-transposed xt tiles [49, 128, 4, 128]
    with xt[t, p, j, q] = x[128t+q, 128j+p] (stationary operand of mm1).
    This removes all PE transposes / PSUM copies from the main loop; bf16
    halves DMA vs f32 (12.8 MB/core, ~36 us at 358 GB/s).
  - tiles are processed in GROUPS of 4 (one DMA pair per group): the four
    s tiles land in one PSUM bank [128, 4, 32] and the softmax runs once
    per group (1 ACT exp + 1 DVE reduce + 1 DVE recip + 1 broadcast mul),
    amortizing the ~150-300 ns per-instruction engine overheads 4x.
  - mm2 emission lags one group behind mm1 so the PE queue never waits on
    the softmax chain of the tile it just computed.
  - softmax skips max-subtraction (|s| <= ~10 is exp-safe in f32).
  - epilogue per sample: vT = C^T*a_sum + acc fused in one
    scalar_tensor_tensor, PE-transpose to [d, k], ACT square + DVE reduce
    for the intra-norm, global L2 folded analytically (the global norm of
    the intra-normalized matrix is exactly sqrt(512)).
Measured end-to-end relative error vs the f32 reference ~2e-3.
"""
import sys

if '/opt/trn_rl_repo' not in sys.path:
    sys.path.insert(0, '/opt/trn_rl_repo')

from contextlib import ExitStack

import numpy as np

N_PIX = 3136
N_SAMP = 2
N_ROWS = N_PIX * N_SAMP
P = 128
NT = N_ROWS // P      # 49
D = 512
K = 32
DC = D // P           # 4
BOUND_T = N_PIX // P  # 24
BOUND_R = N_PIX - BOUND_T * P  # 64
N_CORES = 8
GROUPS = [(0, 1)] + [(1 + 4 * i, 4) for i in range(12)]  # (t0, sz), sum=49
MAXG = 4

_cache = {}


def _build():
    import concourse.bacc as bacc
    import concourse.mybir as mybir
    import concourse.tile as tile
    from concourse.bass import ts

    F32 = mybir.dt.float32
    BF16 = mybir.dt.bfloat16
    MULT = mybir.AluOpType.mult
    ADD = mybir.AluOpType.add

    nc = bacc.Bacc("TRN2", target_bir_lowering=False, debug=False)

    xb = nc.declare_dram_parameter("xb", [N_ROWS, D], BF16, isOutput=False)
    xt = nc.declare_dram_parameter("xt", [NT, P, DC, P], BF16,
                                   isOutput=False)
    wc = nc.declare_dram_parameter("wc", [D, K], BF16, isOutput=False)
    ct = nc.declare_dram_parameter("ct", [K, D], F32, isOutput=False)
    id32 = nc.declare_dram_parameter("id32", [K, K], F32, isOutput=False)
    ones2 = nc.declare_dram_parameter("ones2", [P, 2], BF16, isOutput=False)
    out = nc.declare_dram_parameter("out", [N_SAMP, DC, P, K], F32,
                                    isOutput=True)
    xb, xt, wc, ct, id32, ones2, out = (xb.ap(), xt.ap(), wc.ap(), ct.ap(),
                                        id32.ap(), ones2.ap(), out.ap())
    xb_r = xb.rearrange("(t p) d -> p t d", p=P)      # [P, NT, D]
    xt_r = xt.rearrange("t p j q -> p t j q")         # [P, NT, DC, P]

    with tile.TileContext(nc) as tc, ExitStack() as ctx:
        consts = ctx.enter_context(tc.tile_pool(name="consts", bufs=1))
        xbpool = ctx.enter_context(tc.tile_pool(name="xbpool", bufs=4))
        xtpool = ctx.enter_context(tc.tile_pool(name="xtpool", bufs=4))
        small = ctx.enter_context(tc.tile_pool(name="small", bufs=4))
        epil = ctx.enter_context(tc.tile_pool(name="epil", bufs=2))
        ps_s = ctx.enter_context(tc.tile_pool(name="ps_s", bufs=3,
                                              space="PSUM"))
        ps_acc = ctx.enter_context(tc.tile_pool(name="ps_acc", bufs=2,
                                                space="PSUM"))
        ps_asum = ctx.enter_context(tc.tile_pool(name="ps_asum", bufs=2,
                                                 space="PSUM"))

        wc_sb = consts.tile([P, DC, K], BF16)
        nc.sync.dma_start(out=wc_sb, in_=wc.rearrange("(c p) k -> p c k", p=P))
        ct_sb = consts.tile([K, D], F32)
        nc.sync.dma_start(out=ct_sb, in_=ct)
        id32_sb = consts.tile([K, K], F32)
        nc.sync.dma_start(out=id32_sb, in_=id32)
        ones_sb = consts.tile([P, 2], BF16)
        nc.sync.dma_start(out=ones_sb, in_=ones2)

        acc = [ps_acc.tile([K, D], F32, name=f"acc{s}", tag="acc")
               for s in range(N_SAMP)]
        asum_ps = [ps_asum.tile([K, 2], F32, name=f"asumps{s}", tag="asum_ps")
                   for s in range(N_SAMP)]
        started = [False, False]

        def epilogue(s):
            # vT = C^T * a_sum + acc, fused on DVE
            vt_sb = epil.tile([K, D], F32, name=f"vt{s}", tag="vt")
            nc.vector.scalar_tensor_tensor(vt_sb, ct_sb,
                                           asum_ps[s][:, 0:1], acc[s][:, :],
                                           op0=MULT, op1=ADD)
            v_ps = ps_s.tile([P, DC, K], F32, name=f"vps{s}", tag="sps")
            for j in range(DC):
                nc.tensor.transpose(v_ps[:, j, :], vt_sb[:, ts(j, P)], id32_sb)
            vsq = epil.tile([P, DC, K], F32, name=f"vsq{s}", tag="vsq")
            nc.scalar.activation(vsq, v_ps,
                                 mybir.ActivationFunctionType.Square)
            ssq = epil.tile([P, DC], F32, name=f"ssq{s}", tag="ssq")
            nc.vector.reduce_sum(ssq, vsq, axis=mybir.AxisListType.X)
            snorm = epil.tile([P, DC], F32, name=f"sn{s}", tag="sn")
            nc.scalar.activation(snorm, ssq,
                                 mybir.ActivationFunctionType.Sqrt,
                                 scale=float(D))
            rmult = epil.tile([P, DC], F32, name=f"rm{s}", tag="rm")
            nc.vector.reciprocal(rmult, snorm)
            v_sb = epil.tile([P, DC, K], F32, name=f"v{s}", tag="v")
            nc.vector.tensor_mul(
                v_sb, v_ps,
                rmult.to_broadcast([P, DC, K]))
            nc.sync.dma_start(out=out[s].rearrange("c p k -> p c k"),
                              in_=v_sb)

        def emit_mm2(t, a4, u, xb_t):
            a_sb = a4[:, u, :]
            if t < BOUND_T:
                parts = [(0, 0, P)]
            elif t == BOUND_T:
                parts = [(0, 0, BOUND_R), (1, BOUND_R, P)]
            else:
                parts = [(1, 0, P)]
            for s, r0, r1 in parts:
                first = not started[s]
                started[s] = True
                last_tile = (t == BOUND_T and s == 0) or \
                            (t == NT - 1 and s == 1)
                nc.tensor.matmul(acc[s][:, :], a_sb[r0:r1, :],
                                 xb_t[r0:r1, u, :],
                                 start=first, stop=last_tile,
                                 skip_group_check=True)
                nc.tensor.matmul(asum_ps[s][:, :], a_sb[r0:r1, :],
                                 ones_sb[r0:r1, :],
                                 start=first, stop=last_tile,
                                 skip_group_check=True)
                if last_tile:
                    epilogue(s)

        pending = []
        for t0, sz in GROUPS:
            xb_t = xbpool.tile([P, MAXG, D], BF16, name="xb_t")
            nc.sync.dma_start(out=xb_t[:, 0:sz, :],
                              in_=xb_r[:, t0:t0 + sz, :])
            xt_t = xtpool.tile([P, MAXG, DC, P], BF16, name="xt_t")
            nc.sync.dma_start(out=xt_t[:, 0:sz, :, :],
                              in_=xt_r[:, t0:t0 + sz, :, :])
            s_ps = ps_s.tile([P, MAXG, K], F32, name="s_ps", tag="sps")
            for u in range(sz):
                for j in range(DC):
                    nc.tensor.matmul(s_ps[:, u, :], xt_t[:, u, j, :],
                                     wc_sb[:, j, :],
                                     start=(j == 0), stop=(j == DC - 1),
                                     skip_group_check=True)
            exp4 = small.tile([P, MAXG, K], F32, name="exp4")
            nc.scalar.activation(exp4[:, 0:sz, :], s_ps[:, 0:sz, :],
                                 mybir.ActivationFunctionType.Exp)
            sum4 = small.tile([P, MAXG], F32, name="sum4")
            nc.vector.reduce_sum(sum4[:, 0:sz], exp4[:, 0:sz, :],
                                 axis=mybir.AxisListType.X)
            rcp4 = small.tile([P, MAXG], F32, name="rcp4")
            nc.vector.reciprocal(rcp4[:, 0:sz], sum4[:, 0:sz])
            a4 = small.tile([P, MAXG, K], BF16, name="a4")
            nc.vector.tensor_mul(
                a4[:, 0:sz, :], exp4[:, 0:sz, :],
                rcp4[:, 0:sz].to_broadcast([P, sz, K]))
            for pend in pending:
                emit_mm2(*pend)
            pending = [(t0 + u, a4, u, xb_t) for u in range(sz)]
        for pend in pending:
            emit_mm2(*pend)

    nc.finalize()
    return nc


def _get_nc():
    if "nc" not in _cache:
        _cache["nc"] = _build()
    return _cache["nc"]


def make_maps(x, Wc, C):
    """Host-side prep: shard over batch, build bf16 xb / pre-transposed xt."""
    import ml_dtypes

    bf16 = ml_dtypes.bfloat16
    x = np.asarray(x, dtype=np.float32)
    wc_h = np.asarray(Wc, dtype=np.float32).astype(bf16)
    ct_h = np.ascontiguousarray(np.asarray(C, dtype=np.float32).T)
    id32 = np.eye(K, dtype=np.float32)
    ones2 = np.ones((P, 2), dtype=bf16)

    B = x.shape[0]
    per = B // N_CORES
    maps = []
    for i in range(N_CORES):
        xs = x[i * per:(i + 1) * per].reshape(N_ROWS, D).astype(bf16)
        # xt[t, p, j, q] = xs[128t+q, 128j+p]
        xtt = np.ascontiguousarray(
            xs.reshape(NT, P, DC, P).transpose(0, 3, 2, 1))
        maps.append({"xb": np.ascontiguousarray(xs), "xt": xtt,
                     "wc": wc_h, "ct": ct_h, "id32": id32, "ones2": ones2})
    return maps


def kernel(x, Wc, C):
    from concourse.bass_utils import run_bass_kernel_spmd

    nc = _get_nc()
    maps = make_maps(x, Wc, C)
    res = run_bass_kernel_spmd(nc, maps, list(range(N_CORES)))
    outs = [r["out"].reshape(N_SAMP, D * K) for r in res.results]
    return np.concatenate(outs, axis=0)


# revision 9
# speedup vs baseline: 1.1532x; 1.1532x over previous
"""NetVLAD Trainium2 Bass kernel, SPMD over 8 NeuronCores.

Contract: kernel(x, Wc, C) takes the FULL inputs
  x  [16, 56, 56, 512] f32, Wc [512, 32] f32, C [512, 32] f32
and returns the FULL output [16, 16384] f32 (matches reference()).

Sharding: data-parallel over batch — 2 samples per core; Wc/C replicated.

v4 design:
  - x is uploaded TWICE, both in bf16: pixel-major xb (moving operand of
    mm2 acc += a^T x) and host-pre-transposed xt tiles with
    xt[t, p, j, q] = x[128t+q, 128j+p] (stationary operand of
    mm1 s = x Wc). No PE transposes or PSUM copies in the main loop; bf16
    halves DMA vs f32 (~13 MB/core, ~36 us at 358 GB/s). xb goes through
    the Sync DMA queue, xt through the GpSimd queue so the two streams
    don't serialize on one issuing engine.
  - each sample is PADDED from 3136 to 3200 pixels (25 tiles of 128) with
    x=0. Pad pixels contribute 0 to a^T x and exactly 1/32 per cluster to
    the softmax weights, so a_sum overcounts by the constant 64/32 = 2.0,
    subtracted for free via the bias of the a_sum PSUM->SBUF copy. This
    keeps every 4-tile group inside one sample: no row-split matmuls.
  - tiles are processed in GROUPS of 4 (one xb/xt DMA pair per group):
    the four s tiles accumulate into one PSUM bank [128, 4, 32] and the
    softmax runs once per group (1 ACT exp + 1 DVE reduce + 1 DVE recip +
    1 broadcast mul), amortizing per-instruction overheads 4x.
  - a_sum also runs once per group: one matmul with lhsT = a4 [128, 4*32]
    and rhs = ones gives the per-(tile-slot, cluster) sums on 128 PSUM
    partitions; a tiny fold matmul (eye(32) tiled 4x) in the epilogue
    reduces the 4 tile-slots.
  - mm2 emission lags mm1 by 2 groups so the PE never waits on the
    softmax chain; softmax skips max-subtraction (|s| <= ~10 is f32-safe).
  - epilogue per sample: vT = C^T*a_sum + acc fused in one
    scalar_tensor_tensor, PE-transpose to [d, k], ACT square + DVE reduce
    for the intra-norm; the global L2 norm of the intra-normalized matrix
    is exactly sqrt(512), folded analytically into the Sqrt scale.
Measured end-to-end relative error vs the f32 reference ~2e-3.
"""
import sys

if '/opt/trn_rl_repo' not in sys.path:
    sys.path.insert(0, '/opt/trn_rl_repo')

from contextlib import ExitStack

import numpy as np

N_PIX = 3136
N_SAMP = 2
P = 128
NTS = 25              # tiles per sample (padded to 3200 px)
NT = NTS * N_SAMP     # 50
N_ROWSP = NT * P      # 6400 padded rows
D = 512
K = 32
DC = D // P           # 4
N_CORES = 8
MAXG = 4
LAG = 2               # groups of mm2 lag

# (sample, first local tile, size); all groups lie inside one sample
GROUPS = []
for _s in range(N_SAMP):
    GROUPS += [(_s, 4 * i, 4) for i in range(6)] + [(_s, 24, 1)]

_cache = {}


def _build():
    import concourse.bacc as bacc
    import concourse.mybir as mybir
    import concourse.tile as tile
    from concourse.bass import ts

    F32 = mybir.dt.float32
    BF16 = mybir.dt.bfloat16
    MULT = mybir.AluOpType.mult
    ADD = mybir.AluOpType.add

    nc = bacc.Bacc("TRN2", target_bir_lowering=False, debug=False)

    xb = nc.declare_dram_parameter("xb", [N_ROWSP, D], BF16, isOutput=False)
    xt = nc.declare_dram_parameter("xt", [NT, P, DC, P], BF16,
                                   isOutput=False)
    wc = nc.declare_dram_parameter("wc", [D, K], BF16, isOutput=False)
    ct = nc.declare_dram_parameter("ct", [K, D], F32, isOutput=False)
    id32 = nc.declare_dram_parameter("id32", [K, K], F32, isOutput=False)
    ones2 = nc.declare_dram_parameter("ones2", [P, 2], BF16, isOutput=False)
    wfold = nc.declare_dram_parameter("wfold", [P, K], F32, isOutput=False)
    out = nc.declare_dram_parameter("out", [N_SAMP, DC, P, K], F32,
                                    isOutput=True)
    xb, xt, wc, ct, id32, ones2, wfold, out = (
        xb.ap(), xt.ap(), wc.ap(), ct.ap(), id32.ap(), ones2.ap(),
        wfold.ap(), out.ap())
    xb_r = xb.rearrange("(t p) d -> p t d", p=P)      # [P, NT, D]
    xt_r = xt.rearrange("t p j q -> p t j q")         # [P, NT, DC, P]

    with tile.TileContext(nc) as tc, ExitStack() as ctx:
        consts = ctx.enter_context(tc.tile_pool(name="consts", bufs=1))
        xbpool = ctx.enter_context(tc.tile_pool(name="xbpool", bufs=6))
        xtpool = ctx.enter_context(tc.tile_pool(name="xtpool", bufs=6))
        small = ctx.enter_context(tc.tile_pool(name="small", bufs=6))
        epil = ctx.enter_context(tc.tile_pool(name="epil", bufs=2))
        ps_s = ctx.enter_context(tc.tile_pool(name="ps_s", bufs=3,
                                              space="PSUM"))
        ps_acc = ctx.enter_context(tc.tile_pool(name="ps_acc", bufs=2,
                                                space="PSUM"))
        ps_asum = ctx.enter_context(tc.tile_pool(name="ps_asum", bufs=2,
                                                 space="PSUM"))

        wc_sb = consts.tile([P, DC, K], BF16)
        nc.sync.dma_start(out=wc_sb, in_=wc.rearrange("(c p) k -> p c k", p=P))
        ct_sb = consts.tile([K, D], F32)
        nc.sync.dma_start(out=ct_sb, in_=ct)
        id32_sb = consts.tile([K, K], F32)
        nc.sync.dma_start(out=id32_sb, in_=id32)
        ones_sb = consts.tile([P, 2], BF16)
        nc.sync.dma_start(out=ones_sb, in_=ones2)
        wfold_sb = consts.tile([P, K], F32)
        nc.sync.dma_start(out=wfold_sb, in_=wfold)

        acc = [ps_acc.tile([K, D], F32, name=f"acc{s}", tag="acc")
               for s in range(N_SAMP)]
        asum_ps = [ps_asum.tile([P, 2], F32, name=f"asumps{s}", tag="asum_ps")
                   for s in range(N_SAMP)]

        def epilogue(s):
            # fold the 4 tile-slot blocks of a_sum and subtract the pad
            # contribution (64 pad px * 1/32 = 2.0 total, 0.5 per block)
            asum_sb = epil.tile([P, 2], F32, name=f"asb{s}", tag="asb")
            nc.scalar.activation(asum_sb, asum_ps[s],
                                 mybir.ActivationFunctionType.Copy,
                                 bias=-0.5)
            asum_f = ps_s.tile([K, 2], F32, name=f"af{s}", tag="sps")
            nc.tensor.matmul(asum_f, wfold_sb, asum_sb,
                             start=True, stop=True, skip_group_check=True)
            # vT = C^T * a_sum + acc, fused on DVE
            vt_sb = epil.tile([K, D], F32, name=f"vt{s}", tag="vt")
            nc.vector.scalar_tensor_tensor(vt_sb, ct_sb, asum_f[:, 0:1],
                                           acc[s][:, :], op0=MULT, op1=ADD)
            v_ps = ps_s.tile([P, DC, K], F32, name=f"vps{s}", tag="sps")
            for j in range(DC):
                nc.tensor.transpose(v_ps[:, j, :], vt_sb[:, ts(j, P)], id32_sb)
            vsq = epil.tile([P, DC, K], F32, name=f"vsq{s}", tag="vsq")
            nc.scalar.activation(vsq, v_ps,
                                 mybir.ActivationFunctionType.Square)
            ssq = epil.tile([P, DC], F32, name=f"ssq{s}", tag="ssq")
            nc.vector.reduce_sum(ssq, vsq, axis=mybir.AxisListType.X)
            snorm = epil.tile([P, DC], F32, name=f"sn{s}", tag="sn")
            nc.scalar.activation(snorm, ssq,
                                 mybir.ActivationFunctionType.Sqrt,
                                 scale=float(D))
            rmult = epil.tile([P, DC], F32, name=f"rm{s}", tag="rm")
            nc.vector.reciprocal(rmult, snorm)
            v_sb = epil.tile([P, DC, K], F32, name=f"v{s}", tag="v")
            nc.vector.tensor_mul(v_sb, v_ps, rmult.to_broadcast([P, DC, K]))
            nc.sync.dma_start(out=out[s].rearrange("c p k -> p c k"),
                              in_=v_sb)

        def emit_mm2(s, tl0, sz, a4, xb_t):
            for u in range(sz):
                tl = tl0 + u
                nc.tensor.matmul(acc[s][:, :], a4[:, u, :], xb_t[:, u, :],
                                 start=(tl == 0), stop=(tl == NTS - 1),
                                 skip_group_check=True)
            nc.tensor.matmul(asum_ps[s][0:sz * K, :], a4[:, 0:sz, :],
                             ones_sb,
                             start=(tl0 == 0), stop=(tl0 + sz == NTS),
                             skip_group_check=True)
            if tl0 + sz == NTS:
                epilogue(s)

        pending = []
        for s, tl0, sz in GROUPS:
            t0 = s * NTS + tl0
            xb_t = xbpool.tile([P, MAXG, D], BF16, name="xb_t")
            nc.sync.dma_start(out=xb_t[:, 0:sz, :],
                              in_=xb_r[:, t0:t0 + sz, :])
            xt_t = xtpool.tile([P, MAXG, DC, P], BF16, name="xt_t")
            nc.gpsimd.dma_start(out=xt_t[:, 0:sz, :, :],
                                in_=xt_r[:, t0:t0 + sz, :, :])
            s_ps = ps_s.tile([P, MAXG, K], F32, name="s_ps", tag="sps")
            for u in range(sz):
                for j in range(DC):
                    nc.tensor.matmul(s_ps[:, u, :], xt_t[:, u, j, :],
                                     wc_sb[:, j, :],
                                     start=(j == 0), stop=(j == DC - 1),
                                     skip_group_check=True)
            exp4 = small.tile([P, MAXG, K], F32, name="exp4")
            nc.scalar.activation(exp4[:, 0:sz, :], s_ps[:, 0:sz, :],
                                 mybir.ActivationFunctionType.Exp)
            sum4 = small.tile([P, MAXG], F32, name="sum4")
            nc.vector.reduce_sum(sum4[:, 0:sz], exp4[:, 0:sz, :],
                                 axis=mybir.AxisListType.X)
            rcp4 = small.tile([P, MAXG], F32, name="rcp4")
            nc.vector.reciprocal(rcp4[:, 0:sz], sum4[:, 0:sz])
            a4 = small.tile([P, MAXG, K], BF16, name="a4")
            nc.vector.tensor_mul(
                a4[:, 0:sz, :], exp4[:, 0:sz, :],
                rcp4[:, 0:sz].to_broadcast([P, sz, K]))
            pending.append((s, tl0, sz, a4, xb_t))
            if len(pending) > LAG:
                emit_mm2(*pending.pop(0))
        for pend in pending:
            emit_mm2(*pend)

    nc.finalize()
    return nc


def _get_nc():
    if "nc" not in _cache:
        _cache["nc"] = _build()
    return _cache["nc"]


def make_maps(x, Wc, C):
    """Host-side prep: shard over batch, pad samples to 3200 px, build
    bf16 xb / pre-transposed xt."""
    import ml_dtypes

    bf16 = ml_dtypes.bfloat16
    x = np.asarray(x, dtype=np.float32)
    wc_h = np.asarray(Wc, dtype=np.float32).astype(bf16)
    ct_h = np.ascontiguousarray(np.asarray(C, dtype=np.float32).T)
    id32 = np.eye(K, dtype=np.float32)
    ones2 = np.ones((P, 2), dtype=bf16)
    wfold_h = np.tile(np.eye(K, dtype=np.float32), (DC, 1))

    B = x.shape[0]
    per = B // N_CORES
    maps = []
    for i in range(N_CORES):
        xs = x[i * per:(i + 1) * per].reshape(per, N_PIX, D).astype(bf16)
        xp = np.zeros((per, NTS * P, D), dtype=bf16)
        xp[:, :N_PIX, :] = xs
        xp = xp.reshape(N_ROWSP, D)
        # xt[t, p, j, q] = xp[128t+q, 128j+p]
        xtt = np.ascontiguousarray(
            xp.reshape(NT, P, DC, P).transpose(0, 3, 2, 1))
        maps.append({"xb": np.ascontiguousarray(xp), "xt": xtt,
                     "wc": wc_h, "ct": ct_h, "id32": id32, "ones2": ones2,
                     "wfold": wfold_h})
    return maps


def kernel(x, Wc, C):
    from concourse.bass_utils import run_bass_kernel_spmd

    nc = _get_nc()
    maps = make_maps(x, Wc, C)
    res = run_bass_kernel_spmd(nc, maps, list(range(N_CORES)))
    outs = [r["out"].reshape(N_SAMP, D * K) for r in res.results]
    return np.concatenate(outs, axis=0)
